# revision 27
# baseline (speedup 1.0000x reference)
"""DenseCaptioner LSTM-gate kernel for 8 Trainium2 NeuronCores.

Role-split sharding (halves per-core HBM traffic vs. gate+batch-half
data parallelism):
  cores 0-3  run program VIS: visual + recurrent paths for gate g = core,
             full batch (two 128-row m-tiles)  -> partial logits [256,1024]
  cores 4-7  run program INP: input path for gate g = core-4, full batch
             -> partial logits [256,1024]
Host: logits[g] = vis_part[g] + inp_part[g] + b[g], then sigmoid/tanh gate
math and the prev_c recurrence.

Program structure (bf16 matmuls, fp32 PSUM accumulation):
  - DMA instruction dispatch costs ~0.6us serialized on the sync queue,
    so transfers are laddered [1,1,2,4,8,8...] k-tiles per chunk: small
    first chunks let the PE start ~2us in, big later chunks keep the
    dispatch count low (~60/program)
  - activation chunks are issued inline with their first-use weight
    stream (same k-tile ladder) so act availability tracks weight needs
  - each "gated pair" streams two weight matrices into the two PSUM
    pairs; the Hadamard is a scalar-engine bounce copy + vector mul; the
    transposed copy the next level needs as lhsT is ONE batched DMA-XBAR
    transpose per m-tile (m0 on the scalar DGE queue, m1 deferred on the
    sync queue past the next stream's dispatches to dodge FIFO blocking)
  - vis phase order V-level1 -> U-level1 -> C-level2 -> level3 hides
    junctions under independent matmul streams; U3+C3 share one PSUM
    accumulation so the logits need a single PSUM->SBUF copy at the end
"""

import numpy as np

import jax
from jax.experimental.shard_map import shard_map
from jax.sharding import Mesh, PartitionSpec

import concourse.mybir as mybir
import concourse.tile as tile
from concourse import bacc, bass2jax

B, X, V, MM, VH, H1, H2, G = 256, 12000, 4096, 1024, 1024, 1024, 1024, 4
XP = 12032  # X padded to a multiple of 128 (94 k-tiles)
N_CORES = 8
MT = 2      # m-tiles (batch 256 = 2 x 128)

DT_NAME = "bfloat16"  # matmul dtype: "float32r" or "bfloat16"

# Optional fp8 e4m3 for the huge level-1 weight streams (halves their HBM
# traffic, but e4m3's ~2.4% RMS quantization noise lands final rel err at
# ~3.3e-2, over the 2e-2 gate -- leave empty). e4m3 min normal is 2^-6 and
# these weights have sigma 0.02, so they are pre-scaled by 2^FP8_SHIFT on
# the host; the inverse is folded into next-level bf16 weights.
FP8_NAMES = ()
FP8_SHIFT = 6

_cache = {}


def _mm_dt():
    return getattr(mybir.dt, DT_NAME)


def _np_dt():
    return mybir.dt.np(_mm_dt())


def _w_dt(name):
    return mybir.dt.float8e4 if name in FP8_NAMES else _mm_dt()


def _ladder(total):
    """Chunk sizes in k-tiles: small first for fast PE start, 4-wide after
    (constant size so the DMA stream never falls behind a growth step)."""
    steps, k = [], 0
    for s in (1, 1, 2):
        if k >= total:
            break
        s = min(s, total - k)
        steps.append(s)
        k += s
    while k < total:
        s = min(4, total - k)
        steps.append(s)
        k += s
    return steps


def build_program(role):
    """role "vis": visual+recurrent paths; "inp": input path. Full batch."""
    dt = _mm_dt()
    f32 = mybir.dt.float32
    n_chunk = 512  # max matmul free dim (one PSUM bank)

    nc = bacc.Bacc("TRN2", target_bir_lowering=False, debug=False)

    if role == "vis":
        act_specs = {"v1T": V, "v2T": V, "mT": MM, "hT": H2}
        w_specs = {"V1": V, "V2": V, "C1": VH, "C2": MM, "C3": H1,
                   "U1": H2, "U2": MM, "U3": H1}
    else:
        act_specs = {"xT": XP, "mT": MM}
        w_specs = {"W1": XP, "W2": MM, "W3": H1}

    acts_d = {
        name: nc.dram_tensor(name, [128, k // 128 * B], dt, kind="ExternalInput")
        for name, k in act_specs.items()
    }
    wt = {
        name: nc.dram_tensor(name, [k, H1], _w_dt(name), kind="ExternalInput")
        for name, k in w_specs.items()
    }
    out = nc.dram_tensor("out", [B, H2], dt, kind="ExternalOutput")

    with tile.TileContext(nc) as tc:
        with (
            tc.tile_pool(name="acts", bufs=1) as acts,
            tc.tile_pool(name="wstream", bufs=2) as wstream,
            tc.tile_pool(name="inter", bufs=1) as inter,
            tc.tile_pool(name="ps", bufs=2, space="PSUM") as ps,
        ):
            def act_loader(name):
                """Resident act tile + per-chunk DMA issuer (cols of 256)."""
                ktiles = act_specs[name] // 128
                t = acts.tile([128, ktiles * B], dt, tag=name)
                dram = acts_d[name].ap()

                def load(k0, s):
                    nc.sync.dma_start(
                        t[:, k0 * B:(k0 + s) * B], dram[:, k0 * B:(k0 + s) * B]
                    )
                view4 = t.rearrange("p (t m b) -> p t m b", m=MT, b=128)
                return load, view4

            HW = 512  # column half width (one PSUM bank of fp32)

            def issue_w_half(wname, h, tag_prefix, act_load=None):
                """Laddered DMA chunks for columns [h*512:(h+1)*512] of W."""
                total_kt = w_specs[wname] // 128
                w3 = wt[wname].ap().rearrange("(t p) n -> p t n", p=128)
                chunks, k0 = [], 0
                for ci, s in enumerate(_ladder(total_kt)):
                    if act_load is not None:
                        act_load(k0, s)
                    if tag_prefix == "w":
                        tag, bufs = f"w{s}", (6 if s == 4 else 3)
                    else:
                        tag, bufs = f"{tag_prefix}{h}_{ci}", 1
                    w = wstream.tile([128, s * HW], _w_dt(wname), tag=tag,
                                     bufs=bufs)
                    wv = w.rearrange("p (t n) -> p t n", n=HW)
                    nc.sync.dma_start(
                        wv[:], w3[:, k0:k0 + s, h * HW:(h + 1) * HW]
                    )
                    chunks.append((k0, s, wv))
                    k0 += s
                return chunks

            def issue_w_full(wname, tag_prefix):
                """Full-width laddered chunks (for the level-3 streams)."""
                total_kt = w_specs[wname] // 128
                w3 = wt[wname].ap().rearrange("(t p) n -> p t n", p=128)
                chunks, k0 = [], 0
                for ci, s in enumerate(_ladder(total_kt)):
                    w = wstream.tile([128, s * H1], _w_dt(wname),
                                     tag=f"{tag_prefix}{ci}", bufs=1)
                    wv = w.rearrange("p (t n) -> p t n", n=H1)
                    nc.sync.dma_start(wv[:], w3[:, k0:k0 + s, :])
                    chunks.append((k0, s, wv))
                    k0 += s
                return chunks

            def emit_half(psums, chunks, act, total_kt, start=True, stop=True):
                for (c0, s, wv) in chunks:
                    for t_ in range(s):
                        k = c0 + t_
                        for mi in range(MT):
                            nc.tensor.matmul(
                                psums[mi][:], act(k, mi), wv[:, t_, :],
                                start=start and (k == 0),
                                stop=stop and (k == total_kt - 1),
                            )

            def emit_full(l3h, chunks, act, total_kt, start=True, stop=True):
                for (c0, s, wv) in chunks:
                    for t_ in range(s):
                        k = c0 + t_
                        for mi in range(MT):
                            for h in range(2):
                                nc.tensor.matmul(
                                    l3h[mi][h][:], act(k, mi),
                                    wv[:, t_, h * HW:(h + 1) * HW],
                                    start=start and (k == 0),
                                    stop=stop and (k == total_kt - 1),
                                )

            def gated_pair(wa, act_a, la, wb, act_b, lb,
                           tpa="w", tpb="w", pre_lates=None):
                """q^T image of (actA @ WA) * (actB @ WB), streamed in column
                halves: half-0's hadamard + batched DMA-XBAR transpose run
                under half-1's matmuls, and the PSUM ring (4 one-bank tiles
                per tag) lets the next stream start without waiting for this
                pair's last hadamard. Returns (qT views, h1 xbar closures for
                the caller to dispatch after the next phase's first chunks).
                pre_lates: previous pair's h1 xbars, dispatched right after
                this pair's first chunk issuance."""
                kta = w_specs[wa] // 128
                ktb = w_specs[wb] // 128
                qs, qTvs = [], []
                for mi in range(MT):
                    q = inter.tile([128, H1], dt, tag="q", bufs=3)
                    qs.append(q)
                    qT = inter.tile([128, (H1 // 128) * 128], dt, tag="qT",
                                    bufs=4)
                    qTvs.append(qT.rearrange("p (t b) -> p t b", b=128))

                def hadamard(pa, pb, h):
                    outs = []
                    for mi in range(MT):
                        bounce = inter.tile([128, HW], f32, tag="bounce",
                                            bufs=4)
                        nc.scalar.activation(
                            bounce[:], pb[mi][:],
                            mybir.ActivationFunctionType.Copy,
                        )
                        nc.vector.tensor_mul(
                            qs[mi][:, h * HW:(h + 1) * HW], pa[mi][:],
                            bounce[:],
                        )
                        outs.append(
                            lambda qTv=qTvs[mi], q=qs[mi], h=h:
                            nc.sync.dma_start(
                                qTv[:, h * 4:(h + 1) * 4, :],
                                q[:, h * HW:(h + 1) * HW], transpose=True,
                            )
                        )
                    return outs

                # half 0
                ca = issue_w_half(wa, 0, tpa, la)
                if pre_lates:
                    for f in pre_lates:
                        f()
                cb = issue_w_half(wb, 0, tpb, lb)
                pa = [ps.tile([128, HW], f32, tag="s1", bufs=4,
                              name=f"pa{wa}0_{i}") for i in range(MT)]
                emit_half(pa, ca, act_a, kta)
                pb = [ps.tile([128, HW], f32, tag="s2", bufs=4,
                              name=f"pb{wb}0_{i}") for i in range(MT)]
                emit_half(pb, cb, act_b, ktb)
                x0 = hadamard(pa, pb, 0)
                # half 1: h0's transposes dispatch between the two chunk sets
                ca = issue_w_half(wa, 1, tpa)
                for f in x0:
                    f()
                cb = issue_w_half(wb, 1, tpb)
                pa = [ps.tile([128, HW], f32, tag="s1", bufs=4,
                              name=f"pa{wa}1_{i}") for i in range(MT)]
                emit_half(pa, ca, act_a, kta)
                pb = [ps.tile([128, HW], f32, tag="s2", bufs=4,
                              name=f"pb{wb}1_{i}") for i in range(MT)]
                emit_half(pb, cb, act_b, ktb)
                return qTvs, hadamard(pa, pb, 1)

            def l3_tiles():
                return [[ps.tile([128, HW], f32, tag="s2", bufs=4,
                                 name=f"l3_{mi}_{h}") for h in range(2)]
                        for mi in range(MT)]

            if role == "vis":
                ldv1, v1v = act_loader("v1T")
                ldv2, v2v = act_loader("v2T")
                ldm, mv = act_loader("mT")
                ldh, hv = act_loader("hT")

                qv, lv = gated_pair(
                    "V1", lambda k, mi: v1v[:, k, mi, :], ldv1,
                    "V2", lambda k, mi: v2v[:, k, mi, :], ldv2)
                qu, lu = gated_pair(
                    "U1", lambda k, mi: hv[:, k, mi, :], ldh,
                    "U2", lambda k, mi: mv[:, k, mi, :], ldm,
                    pre_lates=lv)
                qc, lc = gated_pair(
                    "C1", lambda k, mi: qv[mi][:, k, :], None,
                    "C2", lambda k, mi: mv[:, k, mi, :], None,
                    tpa="ca", tpb="cb", pre_lates=lu)

                l3h = l3_tiles()
                u3c = issue_w_full("U3", "ua")
                for f in lc:
                    f()
                c3c = issue_w_full("C3", "ub")
                emit_full(l3h, u3c, lambda k, mi: qu[mi][:, k, :], H1 // 128,
                          start=True, stop=False)
                emit_full(l3h, c3c, lambda k, mi: qc[mi][:, k, :], H1 // 128,
                          start=False, stop=True)
            else:
                ldx, xv = act_loader("xT")
                ldm, mv = act_loader("mT")

                qx, lx = gated_pair(
                    "W1", lambda k, mi: xv[:, k, mi, :], ldx,
                    "W2", lambda k, mi: mv[:, k, mi, :], ldm)

                l3h = l3_tiles()
                w3c = issue_w_full("W3", "ub")
                for f in lx:
                    f()
                emit_full(l3h, w3c, lambda k, mi: qx[mi][:, k, :], H1 // 128)

            out_v = out.ap().rearrange("(m p) n -> m p n", p=128)
            for mi in range(MT):
                o = inter.tile([128, H2], dt, tag="osb", bufs=2)
                for h in range(2):
                    nc.vector.tensor_copy(
                        o[:, h * HW:(h + 1) * HW], l3h[mi][h][:]
                    )
                nc.sync.dma_start(out_v[mi], o[:])

    nc.compile()
    return nc


def _make_runner(nc, devices):
    """Adapted from concourse.bass2jax.run_bass_via_pjrt: same lowering,
    but runs on an explicit device subset and returns unmaterialized jax
    arrays so two programs can be dispatched concurrently."""
    bass2jax.install_neuronx_cc_hook()

    assert nc.dbg_addr is None
    partition_name = (
        nc.partition_id_tensor.name if nc.partition_id_tensor else None
    )

    in_names, out_names, out_avals, zero_outs = [], [], [], []
    for alloc in nc.m.functions[0].allocations:
        if not isinstance(alloc, mybir.MemoryLocationSet):
            continue
        name = alloc.memorylocations[0].name
        if alloc.kind == "ExternalInput":
            if name != partition_name:
                in_names.append(name)
        elif alloc.kind == "ExternalOutput":
            shape = tuple(alloc.tensor_shape)
            dtype = mybir.dt.np(alloc.dtype)
            out_names.append(name)
            out_avals.append(jax.core.ShapedArray(shape, dtype))
            zero_outs.append(np.zeros(shape, dtype))
    n_params = len(in_names)
    n_outs = len(out_avals)
    in_names.extend(out_names)
    if partition_name is not None:
        in_names.append(partition_name)
    donate = tuple(range(n_params, n_params + n_outs))

    def _body(*args):
        operands = list(args)
        if partition_name is not None:
            operands.append(bass2jax.partition_id_tensor())
        outs = bass2jax._bass_exec_p.bind(
            *operands,
            out_avals=tuple(out_avals),
            in_names=tuple(in_names),
            out_names=tuple(out_names),
            lowering_input_output_aliases=(),
            sim_require_finite=True,
            sim_require_nnan=True,
            nc=nc,
        )
        return tuple(outs)

    n_cores = len(devices)
    mesh = Mesh(np.asarray(devices), ("core",))
    in_specs = (PartitionSpec("core"),) * (n_params + n_outs)
    out_specs = (PartitionSpec("core"),) * n_outs
    sharded = jax.jit(
        shard_map(
            _body, mesh=mesh, in_specs=in_specs, out_specs=out_specs,
            check_rep=False,
        ),
        donate_argnums=donate,
        keep_unused=True,
    )

    def run(in_maps):
        assert len(in_maps) == n_cores
        concat_in = [
            np.concatenate(
                [np.asarray(in_maps[c][name]) for c in range(n_cores)], axis=0
            )
            for name in in_names[:n_params]
        ]
        concat_zeros = [
            np.zeros((n_cores * z.shape[0], *z.shape[1:]), z.dtype)
            for z in zero_outs
        ]
        out_arrs = sharded(*concat_in, *concat_zeros)
        return out_names, out_avals, out_arrs

    return run


def _tile_actT(a, kdim):
    """[256 batch, K<=kdim] -> SBUF image [128, (kdim/128) * 256]:
    (p, (t*2+mi)*128+b) = a[mi*128+b, t*128+p], contiguous per partition."""
    ktiles = kdim // 128
    a = np.asarray(a, np.float32)
    if a.shape[1] < kdim:
        a = np.pad(a, ((0, 0), (0, kdim - a.shape[1])))
    # [2m, 128b, ktiles, 128p] -> [128p, ktiles, 2m, 128b]
    r = a.reshape(MT, 128, ktiles, 128).transpose(3, 2, 0, 1)
    return np.ascontiguousarray(r.reshape(128, ktiles * B), dtype=_np_dt())


def kernel(prev_h, prev_c, x, m, v1, v2, V1, V2, C1, C2, C3, W1, W2, W3, U1, U2, U3, b):
    npdt = _np_dt()
    if "runners" not in _cache:
        devs = jax.devices()
        nc_vis = build_program("vis")
        nc_inp = build_program("inp")
        _cache["runners"] = (
            _make_runner(nc_vis, devs[0:4]),
            _make_runner(nc_inp, devs[4:8]),
        )
        _cache["ncs"] = (nc_vis, nc_inp)
    run_vis, run_inp = _cache["runners"]

    v1T_img = _tile_actT(v1, V)
    v2T_img = _tile_actT(v2, V)
    mT_img = _tile_actT(m, MM)
    hT_img = _tile_actT(prev_h, H2)
    xT_img = _tile_actT(x, XP)

    scale = float(2 ** FP8_SHIFT)

    def cast_w(name, arr, fold=1.0):
        arr = np.asarray(arr, np.float32) * fold
        if name in FP8_NAMES:
            arr = arr * scale
        return np.ascontiguousarray(arr).astype(mybir.dt.np(_w_dt(name)))

    c1_fold = 1.0 / (scale * scale) if "V1" in FP8_NAMES else 1.0
    w3_fold = 1.0 / scale if "W1" in FP8_NAMES else 1.0

    vis_maps, inp_maps = [], []
    for g in range(G):
        vis_maps.append({
            "v1T": v1T_img, "v2T": v2T_img, "mT": mT_img, "hT": hT_img,
            "V1": cast_w("V1", V1[g]), "V2": cast_w("V2", V2[g]),
            "C1": cast_w("C1", C1[g], c1_fold),
            "C2": cast_w("C2", C2[g]), "C3": cast_w("C3", C3[g]),
            "U1": cast_w("U1", U1[g]), "U2": cast_w("U2", U2[g]),
            "U3": cast_w("U3", U3[g]),
        })
        w1_pad = np.zeros((XP, H1), np.float32)
        w1_pad[:X] = np.asarray(W1[g], np.float32)
        inp_maps.append({
            "xT": xT_img, "mT": mT_img,
            "W1": cast_w("W1", w1_pad),
            "W2": cast_w("W2", W2[g]),
            "W3": cast_w("W3", W3[g], w3_fold),
        })

    _cache["last_in_maps"] = (vis_maps, inp_maps)

    # dispatch both programs; they run concurrently on disjoint cores
    vnames, vavals, vouts = run_vis(vis_maps)
    inames, iavals, iouts = run_inp(inp_maps)

    vis_out = np.asarray(vouts[0]).astype(np.float32).reshape(G, B, H2)
    inp_out = np.asarray(iouts[0]).astype(np.float32).reshape(G, B, H2)

    logits = vis_out + inp_out + np.asarray(b, np.float32)[:, None, :]

    def sigmoid(z):
        return 1.0 / (1.0 + np.exp(-z))

    i = sigmoid(logits[0])
    f = sigmoid(logits[1])
    o = sigmoid(logits[2])
    cg = np.tanh(logits[3])
    prev_c = np.asarray(prev_c, np.float32)
    new_c = f * prev_c + i * cg
    new_h = o * np.tanh(prev_c)
    return new_h.astype(np.float32), new_c.astype(np.float32)
